# revision 2
# baseline (speedup 1.0000x reference)
"""Alpha-beta filter as a distributed Bass kernel on 8 TRN2 NeuronCores.

The recurrence
    pred = L + V; L' = pred + a*(x - pred); V' = V + b*(L' - L - V)
is linear time-invariant per (b, c):  s_t = M_c s_{t-1} + w_c x_t  with
    M = [[1-a, 1-a], [-ab, 1-ab]],  w = [a, ab],  s_0 = [x_0, 0]
and out_t = L_t = e1^T s_t.  Within a K=128 chunk,
    out[gK+j] = sum_{i<=j} f_{j-i} x[gK+i]  +  e1^T M^{j+1} s_{gK-1}
with taps f_d = e1^T M^d w.  The first term is a per-channel lower-
triangular Toeplitz matmul on the TensorEngine; the second is a RANK-2
correction from the 2-dim IIR state at the chunk boundary, which the
HOST computes exactly (a [2,K] matmul per chunk plus a 2x2 scan over
the 32 chunks) and adds after dequantization.  This halves both the PE
work and the weight traffic vs. materializing the cross-chunk Toeplitz
block (whose exact value is U @ W + O(lambda^128), lambda <= ~0.92 --
i.e. numerically rank 2).

Channels are independent, so the 8 cores split C = 512 into 64 channels
each; per-channel weights are host-precomputed.

Device IO is int8 on both sides (the rel-err budget is 2e-2; symmetric
int8 with a ~4.4-sigma clip costs ~1% on each side): x is quantized on
the host, dequantized to fp16 integers by DVE copies (issued two tiles
ahead of the matmuls), and the PSUM eviction converts fp32 -> int8
directly with the per-channel output scale folded into the weights
(round-to-nearest-even + saturation in the conversion).  Eviction is
split DVE/ACT so both engines stay under the input-DMA pacing; input x
DMAs ride the Sync HWDGE ring, weights the Scalar HWDGE ring,
steady-state output DMAs the GpSimd SWDGE ring (third DMA queue), and
the last tile drains in quarters over both HWDGE rings to shorten the
tail.
"""

import numpy as np

import concourse.mybir as mybir
import concourse.tile as tile
from concourse import bacc
from concourse.bass_utils import run_bass_kernel_spmd

B, T, C = 32, 4096, 512
K = 128                # chunk length == matmul contraction dim
G = T // K             # 32 chunks
NCORES = 8
C_SH = C // NCORES     # 64 channels per core
NCH = 8                # channels per tile iteration
NT = C_SH // NCH       # 8 tile iterations per core
COLS = G * B           # 1024 matmul columns per channel (col = g*B + b)
FREE = NCH * COLS      # 8192 free elems per x tile
CLAMP_LO, CLAMP_HI = 1e-4, 1.0 - 1e-4

DT_W = mybir.dt.float16
NP_W = np.float16

X_CLIP = 4.4           # x ~ N(0,1): int8 clip at 4.4 sigma
O_CLIP = 4.6           # out clip at 4.6 per-channel sigma
S_X = 127.0 / X_CLIP


def _filter_mats(logit_a, logit_b):
    """Per-channel taps f[d,c] (d<K), boundary maps U, Wh, M^K, M^{K-1}(e1-w).

    f[d]   = e1^T M^d w                      (device Toeplitz taps)
    U[j]   = e1^T M^{j+1}                    [K, 2, C]  (out correction)
    Wh[:,i]= M^{K-1-i} w                     [2, K, C]  (chunk -> end state)
    MK     = M^K                             [2, 2, C]  (state scan)
    E0     = M^{K-1} (e1 - w)                [2, C]     (chunk-0 x0 term)
    """
    a = np.clip(1.0 / (1.0 + np.exp(-logit_a.astype(np.float64))), CLAMP_LO, CLAMP_HI)
    b = np.clip(1.0 / (1.0 + np.exp(-logit_b.astype(np.float64))), CLAMP_LO, CLAMP_HI)
    ab = a * b
    M = np.zeros((2, 2, C))
    M[0, 0] = 1 - a
    M[0, 1] = 1 - a
    M[1, 0] = -ab
    M[1, 1] = 1 - ab
    w = np.stack([a, ab])                       # [2, C]

    f = np.zeros((K, C))
    v = w.copy()
    for d in range(K):
        f[d] = v[0]
        v = np.einsum("ijc,jc->ic", M, v)

    U = np.zeros((K, 2, C))
    row = np.stack([np.ones(C), np.zeros(C)])   # e1^T
    for j in range(K):
        row = np.einsum("jc,jkc->kc", row, M)
        U[j] = row

    Wh = np.zeros((2, K, C))
    cur = w.copy()
    for i in range(K - 1, -1, -1):
        Wh[:, i] = cur
        cur = np.einsum("ijc,jc->ic", M, cur)

    eye = np.eye(2)[:, :, None] * np.ones(C)
    MK1 = eye
    for _ in range(K - 1):
        MK1 = np.einsum("ijc,jkc->ikc", M, MK1)
    MK = np.einsum("ijc,jkc->ikc", M, MK1)
    E0 = np.einsum("ijc,jc->ic", MK1, np.stack([1 - a, -ab]))
    return f, U, Wh, MK, E0


def _out_scale(f):
    """Per-channel int8 scale for the output: 127 / (O_CLIP * steady sigma)."""
    sigma = np.sqrt(np.sum(f.astype(np.float64) ** 2, axis=0))  # [C]
    return 127.0 / (O_CLIP * sigma)


def _pack_weights(f, s_out):
    """Build lhsT weight tensors per core with the io scales folded in.

    w0T[i, j, c] = f[j-i, c] (j >= i, else 0), scaled by s_out[c] / S_X so
    PSUM holds out * s_out ready for int8.  Returns per-core [NT, K, NCH*K].
    """
    g = f * (s_out / S_X)[None, :]
    ii = np.arange(K)[:, None]
    jj = np.arange(K)[None, :]
    d0 = jj - ii
    w0T = np.where((d0 >= 0)[:, :, None], g[np.clip(d0, 0, None)], 0.0)  # [i, j, c]
    w_cores = []
    for core in range(NCORES):
        c0 = core * C_SH
        w0c = w0T[:, :, c0 : c0 + C_SH].transpose(2, 0, 1)  # [C_SH, i, j]
        # -> [NT, i, NCH, j] -> [NT, K, NCH*K]
        w0c = w0c.reshape(NT, NCH, K, K).transpose(0, 2, 1, 3).reshape(NT, K, NCH * K)
        w_cores.append(np.ascontiguousarray(w0c).astype(NP_W))
    return w_cores


def _pack_x(xq, core):
    """xq[B, T, C] int8 -> per-core [NT, K(j), NCH(cc) x G(g) x B(b)]."""
    c0 = core * C_SH
    xs = xq[:, :, c0 : c0 + C_SH]                    # [b, t, c]
    xs = xs.reshape(B, G, K, NT, NCH)                # [b, g, j, ct, cc]
    xd = xs.transpose(3, 2, 4, 1, 0)                 # [ct, j, cc, g, b]
    return np.ascontiguousarray(xd.reshape(NT, K, FREE))


def _boundary_correction(x, U, Wh, MK, E0):
    """Exact rank-2 cross-chunk term: corr[b, t, c] = e1^T M^{j+1} s_{gK-1}.

    s_{gK-1} (the IIR state entering chunk g) is scanned on the host:
        s(end of chunk 0) = Wh @ x_chunk0 + E0 * x_0
        s(end of chunk g) = MK @ s_prev + Wh @ x_chunk_g
    Chunk 0's correction is the classic x0 term U[:,0] * x_0 (s_0 = [x0, 0],
    minus the spurious f_j x_0 the device adds -- the identity
    (M^{j+1})_00 = (M^j)_00 - f_j makes both corrections the same formula).
    """
    xcf = x.reshape(B, G, K, C).astype(np.float32)
    z = np.einsum("skc,bgkc->bgsc", Wh.astype(np.float32), xcf)  # [b,g,2,c]
    x0 = xcf[:, 0, 0, :]                                         # [b,c]
    s = z[:, 0] + np.einsum("ic,bc->bic", E0.astype(np.float32), x0)
    s_in = np.zeros((B, G, 2, C), dtype=np.float32)
    for g in range(1, G):
        s_in[:, g] = s
        if g < G - 1:
            s = np.einsum("ijc,bjc->bic", MK.astype(np.float32), s) + z[:, g]
    corr = np.einsum("jsc,bgsc->bgjc", U.astype(np.float32), s_in)
    corr[:, 0] = np.einsum("jc,bc->bjc", U[:, 0, :].astype(np.float32), x0)
    return corr.reshape(B, T, C)


def _unpack_out(od_list, s_out, corr):
    """Inverse of _pack_x for the int8 outputs of all cores -> [B, T, C] f32."""
    out = np.empty((B, T, C), dtype=np.float32)
    inv = (1.0 / s_out).astype(np.float32)
    for core, od in enumerate(od_list):
        c0 = core * C_SH
        o = od.astype(np.float32).reshape(NT, K, NCH, G, B).transpose(4, 3, 1, 0, 2)
        out[:, :, c0 : c0 + C_SH] = o.reshape(B, T, C_SH) * inv[None, None,
                                                                c0 : c0 + C_SH]
    out += corr
    return out


def _build_graph():
    nc = bacc.Bacc("TRN2", debug=False, num_devices=NCORES)
    x_ext = nc.dram_tensor("x", [NT, K, FREE], mybir.dt.int8, kind="ExternalInput")
    w_ext = nc.dram_tensor("w", [NT, K, NCH * K], DT_W, kind="ExternalInput")
    out_ext = nc.dram_tensor("out", [NT, K, FREE], mybir.dt.int8, kind="ExternalOutput")
    xap, wap, oap = (h.ap() for h in (x_ext, w_ext, out_ext))

    with tile.TileContext(nc) as tc:
        with (
            tc.tile_pool(name="xq", bufs=3) as xqp,
            tc.tile_pool(name="xf", bufs=3) as xfp,
            tc.tile_pool(name="op", bufs=3) as op,
            tc.tile_pool(name="wp", bufs=3) as wp,
            tc.tile_pool(name="psum", bufs=4, space="PSUM") as pp,
        ):
            H = FREE // 2

            def tile_front(t):
                """Input DMAs + dequant for tile t (int8 -> fp16 ints).

                Called two tiles ahead so the dequant clears its engine
                FIFO long before the PE needs it.  DMA issue costs ~0.6us
                per dma_start on the issuing engine, so pieces are few:
                tile 0 splits x as [channel 0 | rest] to let the PE start
                as early as possible; every other tile is one piece.
                """
                xq = xqp.tile([K, FREE], mybir.dt.int8, tag="xq")
                wt = wp.tile([K, NCH * K], DT_W, tag="w")
                xf = xfp.tile([K, FREE], mybir.dt.float16, tag="xf")
                if t == 0:
                    E = COLS  # channel 0 first
                    nc.scalar.dma_start(wt[:, 0:K], wap[t][:, 0:K])
                    nc.sync.dma_start(xq[:, 0:E], xap[t][:, 0:E])
                    nc.vector.tensor_copy(xf[:, 0:E], xq[:, 0:E])
                    nc.scalar.dma_start(wt[:, K:], wap[t][:, K:])
                    nc.sync.dma_start(xq[:, E:FREE], xap[t][:, E:FREE])
                    nc.vector.tensor_copy(xf[:, E:H], xq[:, E:H])
                    nc.vector.tensor_copy(xf[:, H:FREE], xq[:, H:FREE])
                else:
                    nc.sync.dma_start(xq[:], xap[t])
                    nc.scalar.dma_start(wt[:], wap[t])
                    nc.vector.tensor_copy(xf[:, 0:H], xq[:, 0:H])
                    nc.vector.tensor_copy(xf[:, H:FREE], xq[:, H:FREE])
                return xf, wt

            fronts = {0: tile_front(0), 1: tile_front(1)}
            for t in range(NT):
                xf, wt = fronts.pop(t)
                last = t == NT - 1
                ot = op.tile([K, FREE], mybir.dt.int8, tag="o")
                for c in range(NCH):
                    ps = pp.tile([K, COLS], mybir.dt.float32, tag="ps")
                    o = c * COLS
                    lhs0 = wt[:, c * K : (c + 1) * K]
                    # current-chunk Toeplitz (two PSUM banks); the
                    # cross-chunk rank-2 term is added on the host
                    nc.tensor.matmul(ps[:, 0:512], lhs0, xf[:, o : o + 512],
                                     start=True, stop=True)
                    nc.tensor.matmul(ps[:, 512:1024], lhs0,
                                     xf[:, o + 512 : o + 1024],
                                     start=True, stop=True)
                    # evacuate PSUM as int8 (fp32 -> int8 rounds + saturates);
                    # steady state: DVE ~1.5 channels + the dequant CASTs,
                    # ACT the rest, both under the input-DMA pacing.  Last
                    # tile: split every channel across both engines to
                    # halve the drain tail.
                    if last:
                        nc.vector.tensor_copy(ot[:, o : o + 512], ps[:, 0:512])
                        nc.scalar.copy(ot[:, o + 512 : o + COLS], ps[:, 512:1024])
                    elif c == 0:
                        nc.vector.tensor_copy(ot[:, o : o + COLS], ps[:])
                    elif c == 1:
                        nc.vector.tensor_copy(ot[:, o : o + 512], ps[:, 0:512])
                        nc.scalar.copy(ot[:, o + 512 : o + COLS], ps[:, 512:1024])
                    else:
                        nc.scalar.copy(ot[:, o : o + COLS], ps[:])
                # out-DMA via SWDGE on the otherwise-idle GpSimd ring (third
                # DMA queue).  Last tile: quarters on alternating HWDGE
                # rings, each fired as soon as its channels are evicted, so
                # the tail after the final evict is one 256 KB transfer.
                if last:
                    for q in range(4):
                        lo, hi = q * 2 * COLS, (q + 1) * 2 * COLS
                        eng = nc.sync if q % 2 == 0 else nc.scalar
                        eng.dma_start(oap[t][:, lo:hi], ot[:, lo:hi])
                else:
                    nc.gpsimd.dma_start(oap[t], ot[:])
                # prefetch + dequant two tiles ahead, AFTER this tile's
                # out-DMA so a dequant never delays an SWDGE issue
                if t + 2 < NT:
                    fronts[t + 2] = tile_front(t + 2)
    nc.compile()
    return nc


_GRAPH = None


def _get_graph():
    global _GRAPH
    if _GRAPH is None:
        _GRAPH = _build_graph()
    return _GRAPH


def _run(x, logit_a, logit_b, trace=False):
    f, U, Wh, MK, E0 = _filter_mats(np.asarray(logit_a), np.asarray(logit_b))
    s_out = _out_scale(f)
    wc = _pack_weights(f, s_out)
    x = np.asarray(x)
    xq = np.clip(np.rint(x * S_X), -127, 127).astype(np.int8)
    in_maps = [{"x": _pack_x(xq, i), "w": wc[i]} for i in range(NCORES)]
    nc = _get_graph()
    last_err = None
    for attempt in range(3):
        try:
            res = run_bass_kernel_spmd(nc, in_maps, list(range(NCORES)), trace=trace)
            break
        except Exception as e:  # transient NRT/axon device errors
            last_err = e
            import time

            time.sleep(5.0)
    else:
        raise last_err
    corr = _boundary_correction(x, U, Wh, MK, E0)
    out = _unpack_out([res.results[i]["out"] for i in range(NCORES)], s_out, corr)
    return out, res


def kernel(x, logit_a, logit_b):
    out, _ = _run(x, logit_a, logit_b)
    return out
